# revision 12
# baseline (speedup 1.0000x reference)
"""Trainium2 Bass kernel for windowed sparse attention (nn_Attention_17703855194428).

Reference computation (per window w of 128 = B*X*Y, tokens N=294 = L*W1*W2):
    qkv = x_w @ w_qkv.T ; q,k,v heads (8 heads x 32 dim), q scaled by 1/sqrt(32)
    sim = q @ k.T + rel_pos_bias ; masked cols -> -1e9 ; softmax over keys
    out = (attn @ v) @ w_out.T

Sharding: pure data parallel over the 128 independent windows -> 16 windows
per NeuronCore, weights/bias replicated. No collectives.

Kernel layout strategy (per core, per window):
    xT [C=256, N=294] (channels on partitions) ->
    q,k as [E, N] (head-dim on partitions), v as [N, E] (tokens on partitions)
    simT[j, i] = sum_d k[d,j] q[d,i] via 4x row-tiled (K=32) matmuls, psum
    P_T = exp(simT + mask_j) * exp(bias)_T   (ACT exp + DVE/GPSIMD multiply)
    outU.T[hd, i] = sum_j v[j, hd] P_T[j, i] via 4x col-tiled (M=32) matmuls
    rowsum[h, i] broadcast to 32 rows free via col-tiled ones-matmul (M=32)
    out = (outU.T * (1/rowsum)).T @ w_out.T via K=hd matmuls -> [tok, C]
"""

import numpy as np
from contextlib import ExitStack

import concourse.bass as bass
import concourse.bacc as bacc
import concourse.mybir as mybir
from concourse import tile
from concourse.bass_utils import run_bass_kernel_spmd

import ml_dtypes

F32 = mybir.dt.float32
F32R = mybir.dt.float32r
BF16 = mybir.dt.bfloat16
FP16 = mybir.dt.float16
EXP = mybir.ActivationFunctionType.Exp

# Problem constants (hardcoded per harness contract)
B, AGENT, X, Y, WIN, DIM, HEADS, DH = 2, 6, 8, 8, 7, 256, 8, 32
N = AGENT * WIN * WIN            # 294 tokens per window
NWIN = B * X * Y                 # 128 windows
NCORES = 8
WPC = NWIN // NCORES             # 16 windows per core
JC = 98                          # key-chunk size (294 = 3*98)
NJC = 3
SCALE = DH ** -0.5
MASK_NEG = -1e9


def _rel_pos_index(L, Wh, Ww):
    coords = np.stack(np.meshgrid(np.arange(L), np.arange(Wh), np.arange(Ww), indexing="ij"))
    cf = coords.reshape(3, -1)
    rel = cf[:, :, None] - cf[:, None, :]
    rel = rel.transpose(1, 2, 0).astype(np.int64)
    rel[..., 0] += L - 1
    rel[..., 1] += Wh - 1
    rel[..., 2] += Ww - 1
    rel[..., 0] *= (2 * Wh - 1) * (2 * Ww - 1)
    rel[..., 1] *= 2 * Ww - 1
    return rel.sum(-1)  # (N, N) [i, j]


def build_graph(n_wins=WPC):
    nc = bacc.Bacc(None)
    xt_d = nc.declare_dram_parameter("xt", [n_wins, 2, 128, N], FP16, isOutput=False)
    msk_d = nc.declare_dram_parameter("msk", [JC, n_wins * NJC], F32, isOutput=False)
    eb_d = nc.declare_dram_parameter("eb", [JC, HEADS, NJC, N], FP16, isOutput=False)
    wqkv_d = nc.declare_dram_parameter("wqkv", [2, 128, 3 * DIM], FP16, isOutput=False)
    wout_d = nc.declare_dram_parameter("wout", [2, 128, DIM], FP16, isOutput=False)
    out_d = nc.declare_dram_parameter("out", [n_wins, N, DIM], F32, isOutput=True)

    with tile.TileContext(nc) as tc, ExitStack() as ctx:
        cpool = ctx.enter_context(tc.tile_pool(name="consts", bufs=1))
        wpool = ctx.enter_context(tc.tile_pool(name="work", bufs=2))
        # one xt slot per window: slot reuse on DMA-written tiles piles up
        # sync waits beyond what DMA descriptors support
        xpool = ctx.enter_context(tc.tile_pool(name="xin", bufs=n_wins))
        psim = ctx.enter_context(tc.tile_pool(name="psim", bufs=1, space="PSUM"))
        ps1 = ctx.enter_context(tc.tile_pool(name="ps1", bufs=4, space="PSUM"))

        # ---- replicated constants ----
        wqkv_sb = []
        for c in range(2):
            t = cpool.tile([128, 3 * DIM], FP16, tag=f"wqkv{c}")
            nc.sync.dma_start(t[:], wqkv_d[c])
            wqkv_sb.append(t)
        wout_sb = []
        for c in range(2):
            t = cpool.tile([128, DIM], FP16, tag=f"wout{c}")
            nc.sync.dma_start(t[:], wout_d[c])
            wout_sb.append(t)
        eb_sb = cpool.tile([JC, HEADS, NJC, N], FP16, tag="eb")
        nc.sync.dma_start(eb_sb[:], eb_d[:])
        msk_sb = cpool.tile([JC, n_wins * NJC], F32, tag="msk")
        nc.sync.dma_start(msk_sb[:], msk_d[:])
        ones_sb = cpool.tile([JC, 32], FP16, tag="ones")
        nc.vector.memset(ones_sb[:], 1.0)

        # warm-up touches: absorb the one-time const-DMA waits into throwaway
        # instructions so steady-state ops stay within the per-instruction
        # sync-wait budget
        scr_a = cpool.tile([JC, 1], F32, tag="scr_a")
        nc.scalar.copy(scr_a[:], msk_sb[:, 0:1])
        scr_v = cpool.tile([JC, 1], FP16, tag="scr_v")
        nc.vector.tensor_copy(scr_v[:], eb_sb[:, 0, 0, 0:1])

        for w in range(n_wins):
            # ---- load xT ----
            xt_t = []
            for c in range(2):
                t = xpool.tile([128, N], FP16, tag=f"xt{c}")
                nc.sync.dma_start(t[:], xt_d[w, c])
                xt_t.append(t)

            # ---- QKV projections ----
            # q/k: [E,N] chunks; part p covers feature cols 128p..128(p+1)
            qk_sb = []
            for p in range(4):
                ps = ps1.tile([128, 512], F32, tag="b1")
                for c in range(2):
                    nc.tensor.matmul(
                        ps[:, 0:N],
                        lhsT=wqkv_sb[c][:, 128 * p:128 * (p + 1)],
                        rhs=xt_t[c][:],
                        start=(c == 0), stop=(c == 1),
                    )
                t = wpool.tile([128, N], FP16, tag=f"qk{p}")
                if p < 2:
                    nc.scalar.copy(t[:], ps[:, 0:N])
                else:
                    nc.vector.tensor_copy(t[:], ps[:, 0:N])
                qk_sb.append(t)

            # v: [N,E] token-chunks (j on partitions), bf16 for the PV matmul
            v_sb = []
            for j in range(NJC):
                ps = ps1.tile([128, 512], F32, tag="b1")
                for c in range(2):
                    nc.tensor.matmul(
                        ps[0:JC, 0:DIM],
                        lhsT=xt_t[c][:, JC * j:JC * (j + 1)],
                        rhs=wqkv_sb[c][:, 2 * DIM:3 * DIM],
                        start=(c == 0), stop=(c == 1),
                    )
                t = wpool.tile([JC, DIM], FP16, tag=f"v{j}")
                nc.scalar.copy(t[:], ps[0:JC, 0:DIM])
                v_sb.append(t)

            # ---- attention (two groups of 4 heads) ----
            on_sb = []
            for hg in range(2):
                pv = ps1.tile([128, 512], F32, tag="b1")
                rs = ps1.tile([128, 512], F32, tag="b1")
                pts = []
                for jc in range(NJC):
                    smp = psim.tile([128, 2048], F32, tag="sim")
                    for t4 in range(4):
                        nc.tensor.matmul(
                            smp[0:JC, 512 * t4:512 * t4 + N],
                            lhsT=qk_sb[2 + hg][32 * t4:32 * (t4 + 1), JC * jc:JC * (jc + 1)],
                            rhs=qk_sb[hg][32 * t4:32 * (t4 + 1), :],
                            start=True, stop=True,
                            tile_position=(32 * t4, 0),
                        )
                    et = wpool.tile([JC, 4, N], FP16, tag="et")
                    sim_ap = smp[0:JC, :].rearrange("p (t x) -> p t x", t=4)[:, :, 0:N]
                    nc.scalar.activation(
                        et[:], sim_ap, EXP,
                        bias=msk_sb[:, NJC * w + jc:NJC * w + jc + 1],
                    )
                    pt = wpool.tile([JC, 4, N], FP16, tag=f"pt{jc}")
                    eb_ap = eb_sb[:, 4 * hg:4 * (hg + 1), jc, :]
                    # keep each reused slot single-consumer-engine: mixed-engine
                    # WAR deps exceed the per-instruction sync-wait limit
                    nc.vector.tensor_mul(pt[:], et[:], eb_ap)
                    pts.append(pt)
                # per-tile contiguous accumulation groups (safe vs whole-bank
                # has_written clear on start)
                for t4 in range(4):
                    h = 4 * hg + t4
                    for jc in range(NJC):
                        nc.tensor.matmul(
                            pv[32 * t4:32 * (t4 + 1), 0:N],
                            lhsT=v_sb[jc][:, 32 * h:32 * (h + 1)],
                            rhs=pts[jc][:, t4, :],
                            start=(jc == 0), stop=(jc == NJC - 1),
                            tile_position=(0, 32 * t4),
                            skip_group_check=True,
                        )
                    for jc in range(NJC):
                        nc.tensor.matmul(
                            rs[32 * t4:32 * (t4 + 1), 0:N],
                            lhsT=ones_sb[:],
                            rhs=pts[jc][:, t4, :],
                            start=(jc == 0), stop=(jc == NJC - 1),
                            tile_position=(0, 32 * t4),
                            skip_group_check=True,
                        )
                rr = wpool.tile([128, N], F32, tag="rr")
                nc.vector.reciprocal_approx_fast(rr[:], rs[:, 0:N])
                on = wpool.tile([128, N], FP16, tag=f"on{hg}")
                nc.vector.tensor_mul(on[:], pv[:, 0:N], rr[:])
                on_sb.append(on)

            # ---- output projection: final[i, e] = sum_hd onT[hd, i] * woutT[hd, e]
            for ic in range(NJC):
                po = ps1.tile([128, 512], F32, tag="b1")
                for kc in range(2):
                    nc.tensor.matmul(
                        po[0:JC, 0:DIM],
                        lhsT=on_sb[kc][:, JC * ic:JC * (ic + 1)],
                        rhs=wout_sb[kc][:],
                        start=(kc == 0), stop=(kc == 1),
                    )
                fo = wpool.tile([JC, DIM], F32, tag="fo")
                nc.vector.tensor_copy(fo[:], po[0:JC, 0:DIM])
                nc.sync.dma_start(out_d[w, JC * ic:JC * (ic + 1), :], fo[:])

    nc.compile()
    return nc


def host_prep(x, mask, w_qkv, w_out, bias_table):
    """Build per-core input maps (numpy only)."""
    x = np.asarray(x, dtype=np.float32)
    mask = np.asarray(mask)
    w_qkv = np.asarray(w_qkv, dtype=np.float32)
    w_out = np.asarray(w_out, dtype=np.float32)
    bias_table = np.asarray(bias_table, dtype=np.float32)

    # x: (B, L, X, Y, W1, W2, C) -> windows (B,X,Y) x [C, N]
    xr = np.ascontiguousarray(x.transpose(0, 2, 3, 1, 4, 5, 6)).reshape(NWIN, N, DIM)
    xt = np.ascontiguousarray(xr.transpose(0, 2, 1)).reshape(NWIN, 2, 128, N).astype(np.float16)

    # mask: (B, X, Y, W1, W2, 1, L) -> (B,X,Y) x N with token order (l, w1, w2)
    m = np.ascontiguousarray(mask.transpose(0, 1, 2, 5, 6, 3, 4)).reshape(NWIN, N)
    maskadd = np.where(m == 0, np.float32(MASK_NEG), np.float32(0.0)).astype(np.float32)

    # exp(bias) transposed: ebT[h, j, i] = exp(bias[i, j, h])
    ri = _rel_pos_index(AGENT, WIN, WIN)
    bias = bias_table[ri]                       # (N, N, H) [i, j, h]
    ebT = np.exp(bias.transpose(2, 1, 0))       # (H, j, i)
    eb_host = np.ascontiguousarray(
        ebT.reshape(HEADS, NJC, JC, N).transpose(2, 0, 1, 3)
    ).astype(np.float16)                # (JC, H, NJC, N)

    wq = w_qkv.copy()
    wq[0:DIM] *= np.float32(SCALE)
    wqkvT = np.ascontiguousarray(wq.T).reshape(2, 128, 3 * DIM).astype(np.float16)
    woutT = np.ascontiguousarray(w_out.T).reshape(2, 128, DIM).astype(np.float16)

    in_maps = []
    for core in range(NCORES):
        ws = slice(WPC * core, WPC * (core + 1))
        mm = maskadd[ws].reshape(WPC, NJC, JC).transpose(2, 0, 1).reshape(JC, WPC * NJC)
        in_maps.append({
            "xt": np.ascontiguousarray(xt[ws]),
            "msk": np.ascontiguousarray(mm),
            "eb": eb_host,
            "wqkv": wqkvT,
            "wout": woutT,
        })
    return in_maps


def assemble_output(core_outs):
    """core_outs: list of [WPC, N, DIM] arrays -> full (B, L, X, Y, W1, W2, C)."""
    out = np.concatenate([np.asarray(o) for o in core_outs], axis=0)  # (NWIN, N, C)
    out = out.reshape(B, X, Y, AGENT, WIN, WIN, DIM)
    return np.ascontiguousarray(out.transpose(0, 3, 1, 2, 4, 5, 6)).astype(np.float32)


_NC_CACHE = {}


def _get_nc(n_wins=WPC):
    if n_wins not in _NC_CACHE:
        _NC_CACHE[n_wins] = build_graph(n_wins)
    return _NC_CACHE[n_wins]


def kernel(x, mask, w_qkv, w_out, bias_table):
    in_maps = host_prep(x, mask, w_qkv, w_out, bias_table)
    nc = _get_nc(WPC)
    res = run_bass_kernel_spmd(nc, in_maps, core_ids=list(range(NCORES)))
    core_outs = [res.results[i]["out"] for i in range(NCORES)]
    return assemble_output(core_outs)
